# revision 8
# baseline (speedup 1.0000x reference)
"""Trainium2 Bass kernel for the CWICDense (conditional stripe matmul) module.

Problem (hardcoded shapes):
  x          [2, 512, 4096] f32    tokens T=1024, features I=4096
  W_kernel   [4096, 4096]   f32    viewed as [I, N=32 stripes, Q=128]
  thresholds [4096, 32]     f32
  mu         [4096]         f32    (structurally zero in this module)
  out_mu     [4096]         f32
  where      [2, 512]       bool   (unused by the reference computation)

  y[t, n*Q+q] = sum_i x_off[t,i] * (|x_off[t,i]| >= thresholds[i,n]) * W[i, n*Q+q]
                + out_mu[n*Q+q]

Sharding across 8 NeuronCores: 2-way data parallel over tokens (halves of
512) x 4-way tensor parallel over stripes (groups of 8 stripes = 1024 out
cols). Each core computes y_c [512, 1024].

Per-core device algorithm:
  - PE-transpose x_c to x^T [I on partitions, T free] (128x128 blocks).
  - a = |x^T| once (DVE tensor_scalar abs_max, exact fp32).
  - per (stripe n, k-tile): one fused scalar_tensor_tensor
      z = (a >= thr[:,n]) * x^T      (per-partition threshold, exact fp32)
    split across DVE and GPSIMD.
  - PE matmul (float32r, N=512 moving) accumulating y^T[n-block] in PSUM
    over 32 k-tiles: acc += W[k,n].T @ z.
  - epilogue: add out_mu (per-partition in y^T layout), PE-transpose back
    to [token, outcol] tiles, DMA out.
"""

import sys

if "/opt/trn_rl_repo" not in sys.path:
    sys.path.insert(0, "/opt/trn_rl_repo")

import numpy as np

import concourse.bass as bass
import concourse.mybir as mybir
import concourse.tile as tile
from concourse import bacc, bass_utils
from concourse.masks import make_identity

# ---- problem constants -------------------------------------------------
B, S, I, N, Q = 2, 512, 4096, 32, 128
T = B * S                 # 1024 tokens
OUT = N * Q               # 4096
NCORES = 8
TOK_WAYS = 2              # token halves
GRP_WAYS = 4              # stripe groups
T_C = T // TOK_WAYS       # 512 tokens per core
NS = N // GRP_WAYS        # 8 stripes per core
OUT_C = NS * Q            # 1024 out cols per core
KT = I // 128             # 32 contraction tiles
P = 128

# z-production split within each stripe's 32 k-tiles: the first DVE_K run as
# one fused scalar_tensor_tensor on DVE; the rest run as DVE mask (2x mode)
# + GPSIMD tensor_tensor multiply, so both engines stay busy throughout.
DVE_K = 18

_CACHE = {}


def _build():
    f32 = mybir.dt.float32
    f32r = mybir.dt.float32r
    nc = bacc.Bacc("TRN2", target_bir_lowering=False, debug=False)

    x_d = nc.dram_tensor("x", [T_C, I], f32, kind="ExternalInput").ap()
    w_d = nc.dram_tensor("w", [I, OUT_C], f32, kind="ExternalInput").ap()
    thr_d = nc.dram_tensor("thr", [I, NS], f32, kind="ExternalInput").ap()
    mu_d = nc.dram_tensor("mu", [P, NS], f32, kind="ExternalInput").ap()
    y_d = nc.dram_tensor("y", [T_C, OUT_C], f32, kind="ExternalOutput").ap()

    ge = mybir.AluOpType.is_ge
    mult = mybir.AluOpType.mult
    absmax = mybir.AluOpType.abs_max
    add = mybir.AluOpType.add

    with tile.TileContext(nc) as tc:
        with (
            tc.tile_pool(name="const", bufs=1) as constp,
            tc.tile_pool(name="xT", bufs=KT) as xTp,
            tc.tile_pool(name="absa", bufs=KT) as ap_,
            tc.tile_pool(name="xnat", bufs=4) as xnatp,
            tc.tile_pool(name="w", bufs=8) as wp,
            tc.tile_pool(name="z", bufs=6) as zp,
            tc.tile_pool(name="m", bufs=4) as mp,
            tc.tile_pool(name="yT", bufs=2) as yTp,
            tc.tile_pool(name="ysb", bufs=2) as ysbp,
            tc.tile_pool(name="tps", bufs=3, space="PSUM") as tpsp,
            tc.tile_pool(name="acc", bufs=2, space="PSUM") as accp,
        ):
            ident = constp.tile([P, P], f32, tag="ident")
            make_identity(nc, ident[:])

            thr_sb = constp.tile([P, KT * NS], f32, tag="thr")
            for k in range(KT):
                nc.sync.dma_start(
                    thr_sb[:, k * NS:(k + 1) * NS], thr_d[k * P:(k + 1) * P, :]
                )
            mu_sb = constp.tile([P, NS], f32, tag="mu")
            nc.sync.dma_start(mu_sb[:], mu_d)

            # ---- phase A: x -> x^T (PE transpose), a = |x^T| ----------
            xT = []
            aT = []
            for k in range(KT):
                xk = xTp.tile([P, T_C], f32, tag="xT")
                ps = tpsp.tile([P, T_C], f32, tag="tps")
                for c in range(T_C // P):
                    xn = xnatp.tile([P, P], f32, tag="xnat")
                    nc.sync.dma_start(
                        xn[:], x_d[c * P:(c + 1) * P, k * P:(k + 1) * P]
                    )
                    nc.tensor.transpose(ps[:, c * P:(c + 1) * P], xn[:], ident[:])
                nc.scalar.copy(xk[:], ps[:])
                ak = ap_.tile([P, T_C], f32, tag="absa")
                nc.scalar.activation(
                    ak[:], xk[:], mybir.ActivationFunctionType.Abs
                )
                xT.append(xk)
                aT.append(ak)

            # ---- phase B: masked stripe matmuls -----------------------
            for n in range(NS):
                acc = accp.tile([P, T_C], f32, tag="acc")
                for k in range(KT):
                    wt = wp.tile([P, Q], f32r, tag="w")
                    nc.sync.dma_start(
                        wt[:],
                        w_d[k * P:(k + 1) * P, n * Q:(n + 1) * Q].bitcast(f32r),
                    )
                    zt = zp.tile([P, T_C], f32r, tag="z")
                    if k < DVE_K:
                        nc.vector.scalar_tensor_tensor(
                            zt[:], aT[k][:],
                            thr_sb[:, k * NS + n:k * NS + n + 1],
                            xT[k][:], op0=ge, op1=mult,
                        )
                    else:
                        mt = mp.tile([P, T_C], f32, tag="m")
                        nc.vector.tensor_scalar(
                            mt[:], aT[k][:],
                            thr_sb[:, k * NS + n:k * NS + n + 1],
                            None, op0=ge,
                        )
                        nc.gpsimd.tensor_tensor(
                            zt[:], mt[:], xT[k][:], op=mult
                        )
                    nc.tensor.matmul(
                        acc[:],
                        wt[:],
                        zt[:],
                        start=(k == 0),
                        stop=(k == KT - 1),
                    )
                # epilogue for stripe n: + out_mu, transpose, store
                yT = yTp.tile([P, T_C], f32, tag="yT")
                nc.vector.tensor_scalar(
                    yT[:], acc[:], mu_sb[:, n:n + 1], None, op0=add
                )
                ps2 = tpsp.tile([P, T_C], f32, tag="tps")
                for c in range(T_C // P):
                    nc.tensor.transpose(
                        ps2[:, c * P:(c + 1) * P], yT[:, c * P:(c + 1) * P],
                        ident[:],
                    )
                ysb = ysbp.tile([P, T_C], f32, tag="ysb")
                nc.scalar.copy(ysb[:], ps2[:])
                for c in range(T_C // P):
                    nc.sync.dma_start(
                        y_d[c * P:(c + 1) * P, n * Q:(n + 1) * Q],
                        ysb[:, c * P:(c + 1) * P],
                    )
    nc.compile()
    return nc


def _get_nc():
    if "nc" not in _CACHE:
        _CACHE["nc"] = _build()
    return _CACHE["nc"]


def _make_in_maps(x, W_kernel, thresholds, mu, out_mu):
    xf = np.ascontiguousarray(x, dtype=np.float32).reshape(T, I)
    xf = xf - np.asarray(mu, dtype=np.float32)[None, :]
    in_maps = []
    for core in range(NCORES):
        h, g = divmod(core, GRP_WAYS)
        mu_c = np.ascontiguousarray(
            np.asarray(out_mu, dtype=np.float32)[g * OUT_C:(g + 1) * OUT_C]
            .reshape(NS, P).T
        )
        in_maps.append({
            "x": np.ascontiguousarray(xf[h * T_C:(h + 1) * T_C]),
            "w": np.ascontiguousarray(
                np.asarray(W_kernel, np.float32)[:, g * OUT_C:(g + 1) * OUT_C]
            ),
            "thr": np.ascontiguousarray(
                np.asarray(thresholds, np.float32)[:, g * NS:(g + 1) * NS]
            ),
            "mu": mu_c,
        })
    return in_maps


def _assemble(results):
    y = np.empty((T, OUT), np.float32)
    for core in range(NCORES):
        h, g = divmod(core, GRP_WAYS)
        y[h * T_C:(h + 1) * T_C, g * OUT_C:(g + 1) * OUT_C] = results[core]["y"]
    return y.reshape(B, S, OUT)


def run(inputs, **spmd_kwargs):
    """Run on hardware; returns (y, BassKernelResults)."""
    nc = _get_nc()
    in_maps = _make_in_maps(
        inputs["x"], inputs["W_kernel"], inputs["thresholds"],
        inputs["mu"], inputs["out_mu"],
    )
    res = bass_utils.run_bass_kernel_spmd(
        nc, in_maps, core_ids=list(range(NCORES)), **spmd_kwargs
    )
    return _assemble(res.results), res


def kernel(x, W_kernel, thresholds, mu, out_mu, where):
    y, _ = run({
        "x": x, "W_kernel": W_kernel, "thresholds": thresholds,
        "mu": mu, "out_mu": out_mu, "where": where,
    })
    return y
